# revision 49
# baseline (speedup 1.0000x reference)
"""MHA (B=2, S=2048, D=1024, H=16) on 8 Trainium2 NeuronCores.

Sharding: core c = (batch b = c//4, head-group g = c%4, 4 heads each).
Per core: Q/K/V projections for its 4 heads (tensor-parallel column split),
full attention for those heads in transposed (scoresT = [k, q]) layout so no
on-chip transposes are needed, then a per-head 8-way AllToAll redistributes
outputs so each core owns a 256-row sequence slice of BOTH batches with all
1024 concat-head dims, and runs the output projection for its slice. Host
only concatenates slices.

Schedule: one packed DRAM tensor [xT | wq*scale | wk | wv] streams in as 8
k-slabs; Q/K projections for heads 0/1 accumulate k-outer in PSUM while the
slabs arrive, so attention starts ~20us in. The V projection and the
heads-2/3 Q/K projections run as PE fillers inside the (ACT-engine-bound)
attention pipeline, spread so each phase stays balanced. Everything computes
in bf16 (PSUM accumulation fp32); softmax skips max-subtraction and folds
the row-sum into the attn@V matmul via an all-ones column on V. The AllToAll
payload is bf16; the out-projection pre-accumulates pair-0 k-tiles while the
last AllToAll is in flight.
"""

import numpy as np

B, S, D, H = 2, 2048, 1024, 16
HD = D // H          # 64
G = H // 4           # 4 head-groups
GH = 4               # heads per core
LD = GH * HD         # 256 local dims per core
CORES = 8
QS = S // CORES      # 256: per-core final sequence slice (per batch)
P = 128
KT = D // P          # 8 k-tiles of the model dim
VW = HD + 1          # 65: per-head V width incl. ones column
XW = S + 3 * LD      # 2816: slab row = xT row | wq | wk | wv
HQ = S // 2          # 1024: q-half processed per PSUM accumulator

_CACHE = {}


def _build_nc():
    import concourse.mybir as mybir
    import concourse.tile as tile
    from concourse import bacc

    F32 = mybir.dt.float32
    BF16 = mybir.dt.bfloat16
    EXP = mybir.ActivationFunctionType.Exp
    MUL = mybir.AluOpType.mult
    ADD = mybir.AluOpType.add

    nc = bacc.Bacc("TRN2", target_bir_lowering=False, debug=False,
                   num_devices=CORES)

    d_xw = nc.dram_tensor("xw", [D, XW], BF16, kind="ExternalInput")
    d_bqk = nc.dram_tensor("bqk", [P, 4], F32, kind="ExternalInput")
    d_vibo = nc.dram_tensor("vibo", [1, GH * VW + D], F32, kind="ExternalInput")
    d_wo = nc.dram_tensor("woT", [D, D], BF16, kind="ExternalInput")
    d_y = nc.dram_tensor("y", [B, QS, D], F32, kind="ExternalOutput")

    # slab column offsets
    OQ, OK, OV = S, S + LD, S + 2 * LD

    with tile.TileContext(nc) as tc:
        with (
            tc.tile_pool(name="statics", bufs=1) as st,
            tc.tile_pool(name="dram", bufs=1, space="DRAM") as dram,
        ):
            # zeros tile: lhsT of warm-up / warm-keeping no-op matmuls
            zz = st.tile([P, P], BF16, tag="zz", name="zz")
            nc.gpsimd.memset(zz[:], 0.0)
            # dummy activation so the ACT table load happens at t~1us, not
            # right before the first real exp
            dum = st.tile([1, 4], F32, tag="dum", name="dum")
            nc.scalar.activation(dum[:], zz[0:1, 0:4], EXP)

            xw = [st.tile([P, XW], BF16, tag=f"xw{k}", name=f"xw{k}")
                  for k in range(KT)]
            for k in range(KT):
                nc.sync.dma_start(xw[k][:], d_xw[k * P:(k + 1) * P, :])
            bqk = st.tile([P, 4], F32, tag="bqk", name="bqk")
            nc.sync.dma_start(bqk[:], d_bqk[:])

            vibo = st.tile([1, GH * VW + D], F32, tag="vibo", name="vibo")
            nc.sync.dma_start(vibo[:], d_vibo[:])
            wo = [st.tile([P, D], BF16, tag=f"wo{k}", name=f"wo{k}")
                  for k in range(KT)]
            for k in range(KT):
                nc.sync.dma_start(wo[k][:], d_wo[k * P:(k + 1) * P, :])

            vib = st.tile([P, GH * VW], F32, tag="vib", name="vib")
            nc.gpsimd.partition_broadcast(vib[:], vibo[:, 0:GH * VW])
            bob = st.tile([P, D], F32, tag="bob", name="bob")

            # persistent activations (m index: 0 = heads 0/1, 1 = heads 2/3)
            qm = [st.tile([P, S], BF16, tag=f"qm{t}", name=f"qm{t}") for t in range(2)]
            km = [st.tile([P, S], BF16, tag=f"km{t}", name=f"km{t}") for t in range(2)]
            vaug = [st.tile([P, GH * VW], BF16, tag=f"va{s}", name=f"va{s}")
                    for s in range(S // P)]
            outT = [st.tile([P, S], BF16, tag=f"oT{t}", name=f"oT{t}") for t in range(2)]
            aop = [st.tile([P, CORES * QS], BF16, tag=f"aop{p}", name=f"aop{p}")
                   for p in range(2)]

            a_ins = [dram.tile([CORES * HD, QS], BF16, name=f"a_in{h}")
                     for h in range(GH)]
            a_outs = [dram.tile([CORES * HD, QS], BF16, name=f"a_out{h}")
                      for h in range(GH)]

            # ---- Q/K m0 (heads 0/1) + V s0 projections, k-outer over the
            # streaming slabs (8 PSUM banks: 4 q + 3 k + 1 v; K m0 n3 runs
            # later as a filler, it isn't consumed until head-0 step 12) ----
            with tc.tile_pool(name="pm0", bufs=1, space="PSUM") as pm0:
                qn = [pm0.tile([P, 512], F32, tag=f"qn{n}", name=f"qn{n}")
                      for n in range(4)]
                kn = [pm0.tile([P, 512], F32, tag=f"kn{n}", name=f"kn{n}")
                      for n in range(3)]
                v0 = pm0.tile([P, LD], F32, tag="v0", name="v0")

                def warm(count):
                    # zero-matmuls ramp the PE pstate / keep the busy streak
                    # alive between slab arrivals (results never read)
                    for _ in range(count):
                        nc.tensor.matmul(qn[0][:, 0:P], zz[:], zz[:],
                                         start=True, stop=True)

                warm(60)
                for k in range(KT):
                    for n in range(4):
                        nc.tensor.matmul(
                            qn[n][:], xw[k][:, OQ:OQ + P],
                            xw[k][:, n * 512:(n + 1) * 512],
                            start=(k == 0), stop=(k == KT - 1))
                        if n < 3:
                            nc.tensor.matmul(
                                kn[n][:], xw[k][:, OK:OK + P],
                                xw[k][:, n * 512:(n + 1) * 512],
                                start=(k == 0), stop=(k == KT - 1))
                    nc.tensor.matmul(
                        v0[:], xw[k][:, 0:P], xw[k][:, OV:OV + LD],
                        start=(k == 0), stop=(k == KT - 1))
                # evac on 3 parallel lanes (ACT is idle until attention):
                # DVE k1,q0,q1 + v0; ACT k0,q2,q3 (Identity+bias); Pool k2
                IDENT = mybir.ActivationFunctionType.Identity
                nc.vector.tensor_tensor(
                    km[0][:, 512:1024], kn[1][:],
                    bqk[:, 2:3].to_broadcast((P, 512)), ADD)
                for n in range(2):
                    nc.vector.tensor_tensor(
                        qm[0][:, n * 512:(n + 1) * 512], qn[n][:],
                        bqk[:, 0:1].to_broadcast((P, 512)), ADD)
                nc.scalar.activation(km[0][:, 0:512], kn[0][:], IDENT,
                                     bias=bqk[:, 2:3])
                nc.scalar.activation(km[0][:, 1024:1536], kn[2][:], IDENT,
                                     bias=bqk[:, 2:3])
                for n in (2, 3):
                    nc.scalar.activation(
                        qm[0][:, n * 512:(n + 1) * 512], qn[n][:], IDENT,
                        bias=bqk[:, 0:1])
                va = vaug[0].rearrange("p (h w) -> p h w", w=VW)
                vb = vib.rearrange("p (h w) -> p h w", w=VW)
                nc.vector.tensor_tensor(
                    va[:, :, 0:HD], v0.rearrange("p (h w) -> p h w", w=HD),
                    vb[:, :, 0:HD], ADD)
                nc.vector.tensor_copy(va[:, :, HD:VW], vb[:, :, HD:VW])

            # ---- attention (ACT-bound), with V and Q/K m1 as PE fillers ----
            with (
                tc.tile_pool(name="psc", bufs=2, space="PSUM") as psc,
                tc.tile_pool(name="pav", bufs=1, space="PSUM") as pav,
                tc.tile_pool(name="pj", bufs=1, space="PSUM") as pj,
                tc.tile_pool(name="exp", bufs=4) as xp,
                tc.tile_pool(name="nrm", bufs=2) as nr,
            ):
                def v_chain(s):
                    vt = pv.tile([P, LD], F32, tag="vt", name="vt")
                    for k in range(KT):
                        nc.tensor.matmul(
                            vt[:], xw[k][:, s * P:(s + 1) * P],
                            xw[k][:, OV:OV + LD],
                            start=(k == 0), stop=(k == KT - 1))
                    va = vaug[s].rearrange("p (h w) -> p h w", w=VW)
                    vb = vib.rearrange("p (h w) -> p h w", w=VW)
                    nc.vector.tensor_tensor(
                        va[:, :, 0:HD], vt.rearrange("p (h w) -> p h w", w=HD),
                        vb[:, :, 0:HD], ADD)
                    nc.vector.tensor_copy(va[:, :, HD:VW], vb[:, :, HD:VW])

                # q/k projection chains, emitted as two 4-matmul chunks
                # sharing one PSUM accumulation group
                _jt = {}

                def qk_chunk(which, m, n, c):
                    dst, boff, off = ((qm, m, OQ) if which == 0 else (km, 2 + m, OK))
                    if c == 0:
                        _jt[which, m, n] = pj.tile([P, 512], F32, tag="jt", name="jt")
                    jt = _jt[which, m, n]
                    for k in range(c * 4, c * 4 + 4):
                        nc.tensor.matmul(
                            jt[:], xw[k][:, off + m * P:off + (m + 1) * P],
                            xw[k][:, n * 512:(n + 1) * 512],
                            start=(k == 0), stop=(k == KT - 1))
                    if c == 1:
                        nc.vector.tensor_tensor(
                            dst[m][:, n * 512:(n + 1) * 512], jt[:],
                            bqk[:, boff:boff + 1].to_broadcast((P, 512)), ADD)

                # filler schedule: (h, half, kb) -> list of emit callables.
                # V chains pace head-0 half-0 (vaug[s] ready 2 steps early);
                # k-m0-n3 (consumed from head-0 step 12) lands on steps 0/1;
                # q1n0/q1n1/k1n0 must precede head 2; the rest slots into
                # head 2's ACT-bound slack just before first use.
                sched = {}
                for s in range(2, S // P):
                    sched.setdefault((0, 0, s - 2), []).append(lambda s=s: v_chain(s))
                sched.setdefault((0, 0, 0), []).append(lambda: v_chain(1))
                for c in range(2):
                    sched.setdefault((0, 0, 4 + c), []).append(
                        lambda c=c: qk_chunk(1, 0, 3, c))
                # q1n0/q1n1/k1n0 precede head 2; head 1 is ACT-bound with
                # ~240ns/step of PE slack, so spread the chunks there (the
                # LAG-deep pipeline absorbs them without starving the ACT)
                pre2 = [(0, n, c) for n in range(2) for c in range(2)] + \
                       [(1, 0, 0), (1, 0, 1)]
                for i, (w, n, c) in enumerate(pre2):
                    sched.setdefault((1, 0, 2 * i), []).append(
                        lambda w=w, n=n, c=c: qk_chunk(w, 1, n, c))
                in2 = [(1, 1, 1, 1, 1, 0), (1, 1, 3, 1, 1, 1),
                       (2, 0, 4, 1, 2, 0), (2, 0, 5, 1, 2, 1),
                       (2, 0, 8, 1, 3, 0), (2, 0, 9, 1, 3, 1),
                       (2, 0, 11, 0, 2, 0), (2, 0, 12, 0, 2, 1),
                       (2, 0, 14, 0, 3, 0), (2, 0, 15, 0, 3, 1)]
                for (h, half, kb, w, n, c) in in2:
                    sched.setdefault((h, half, kb), []).append(
                        lambda w=w, n=n, c=c: qk_chunk(w, 1, n, c))

                NKB = S // P
                av_t = {}

                def finish_half(h, half):
                    # normalize: outT_h = av[0:64] / av[64], then stage + A2A
                    ht, hp = h // 2, (h % 2) * HD
                    q0 = half * HQ
                    av = av_t[h, half]
                    last = (h == GH - 1 and half == 1)
                    if not last:
                        # quick evac frees the accumulator banks fast
                        avs = nr.tile([VW, HQ], F32, tag="avs", name="avs")
                        nc.vector.tensor_copy(avs[:], av[:])
                        rr = nr.tile([1, HQ], F32, tag="rr", name="rr")
                        nc.vector.reciprocal(rr[:], avs[HD:VW, :])
                        rrb = nr.tile([HD, HQ], F32, tag="rrb", name="rrb")
                        nc.gpsimd.partition_broadcast(rrb[:], rr[:])
                        nc.vector.tensor_tensor(
                            outT[ht][hp:hp + HD, q0:q0 + HQ],
                            avs[0:HD, :], rrb[:], MUL)
                        # stage this half for the A2A: q columns [q0, q0+HQ)
                        # are the slices of cores 4*half..3+4*half = a_in rows
                        nc.sync.dma_start(
                            a_ins[h][half * 4 * HD:(half + 1) * 4 * HD, :]
                            .rearrange("(j p) c -> p j c", p=HD),
                            outT[ht][hp:hp + HD, q0:q0 + HQ]
                            .rearrange("p (j c) -> p j c", c=QS))
                    else:
                        # latency-critical path into the final A2A: both
                        # recips back-to-back on DVE, broadcasts pipeline on
                        # Pool, then muls + per-chunk stage DMAs
                        rrs, rcs = [], []
                        for cc in range(2):
                            rr = nr.tile([1, 512], F32, tag="rr", name="rr")
                            nc.vector.reciprocal(
                                rr[:], av[HD:VW, cc * 512:(cc + 1) * 512])
                            rrs.append(rr)
                        for cc in range(2):
                            rc = nr.tile([HD, 512], F32, tag="rc", name="rc")
                            nc.gpsimd.partition_broadcast(rc[:], rrs[cc][:])
                            rcs.append(rc)
                        for cc in range(2):
                            c0 = q0 + cc * 512
                            nc.vector.tensor_tensor(
                                outT[ht][hp:hp + HD, c0:c0 + 512],
                                av[0:HD, cc * 512:(cc + 1) * 512],
                                rcs[cc][:], MUL)
                            # j-blocks 2 per 512-col chunk
                            j0 = half * 4 + cc * 2
                            nc.sync.dma_start(
                                a_ins[h][j0 * HD:(j0 + 2) * HD, :]
                                .rearrange("(j p) c -> p j c", p=HD),
                                outT[ht][hp:hp + HD, c0:c0 + 512]
                                .rearrange("p (j c) -> p j c", c=QS))
                    if half == 1:
                        # per-head 8-way AllToAll (overlaps later heads)
                        nc.gpsimd.collective_compute(
                            "AllToAll",
                            mybir.AluOpType.bypass,
                            replica_groups=[list(range(CORES))],
                            ins=[a_ins[h][:]],
                            outs=[a_outs[h][:]],
                        )
                        if h == 1 or h == 3:
                            p = h // 2
                            for hh in range(2):
                                nc.sync.dma_start(
                                    aop[p][hh * HD:(hh + 1) * HD, :].rearrange(
                                        "p (j c) -> p j c", c=QS),
                                    a_outs[2 * p + hh].rearrange(
                                        "(j p) c -> p j c", p=HD))

                def av_mm(h, half, kb, ex):
                    if kb == 0:
                        av_t[h, half] = pav.tile([VW, HQ], F32, tag="av", name="av")
                    av = av_t[h, half]
                    for nn in range(2):
                        nc.tensor.matmul(
                            av[:, nn * 512:(nn + 1) * 512],
                            vaug[kb][:, h * VW:(h + 1) * VW],
                            ex[:, nn * 512:(nn + 1) * 512],
                            start=(kb == 0), stop=(kb == NKB - 1))
                    if kb == NKB - 1:
                        finish_half(h, half)

                # one flat software pipeline across every (head, half, kb):
                # sc/exp run LAG steps ahead of attn@V, so the ACT engine
                # never stalls, including across half/head boundaries where
                # the normalization chain overlaps the next half's scores.
                LAG = 1
                with tc.tile_pool(name="pv", bufs=1, space="PSUM") as pv:
                    exq = []
                    for h in range(GH):
                        ht, hp = h // 2, (h % 2) * HD
                        kTh = km[ht][hp:hp + HD, :]
                        qTh = qm[ht][hp:hp + HD, :]
                        for half in range(2):
                            q0 = half * HQ
                            for kb in range(NKB):
                                sc = psc.tile([P, HQ], F32, tag="sc", name="sc")
                                for nn in range(2):
                                    nc.tensor.matmul(
                                        sc[:, nn * 512:(nn + 1) * 512],
                                        kTh[:, kb * P:(kb + 1) * P],
                                        qTh[:, q0 + nn * 512:q0 + (nn + 1) * 512],
                                        start=True, stop=True)
                                for fn in sched.get((h, half, kb), ()):
                                    fn()
                                ex = xp.tile([P, HQ], BF16, tag="ex", name="ex")
                                nc.scalar.activation(ex[:], sc[:], EXP)
                                exq.append((h, half, kb, ex))
                                if len(exq) > LAG:
                                    av_mm(*exq.pop(0))
                    for entry in exq:
                        av_mm(*entry)

            # ---- out projection for my 256-row slice of each batch ----
            # k-tile k = dims [k*128,(k+1)*128) of batch bb = sender core
            # c' = bb*4 + k//2, its local head pair p = k%2; all 8 chains
            # pre-accumulate their pair-0 k-tiles while pair 1's A2A flies.
            with (
                tc.tile_pool(name="pop", bufs=1, space="PSUM") as pop,
                tc.tile_pool(name="yst", bufs=2) as yst,
            ):
                nc.gpsimd.partition_broadcast(bob[:], vibo[:, GH * VW:])
                chains = [(bb, m, n) for bb in range(B)
                          for m in range(QS // P) for n in range(2)]
                ps = {c: pop.tile([P, 512], F32, tag=f"ps{i}", name=f"ps{i}")
                      for i, c in enumerate(chains)}
                PAD = 70  # warm-keeping no-op matmuls while pair-1's A2A flies
                PRE = 0  # ramp the PE before the pair-0 accumulation
                for pr in range(2):
                    for i in range(PRE if pr == 0 else PAD):
                        bb, m, n = chains[i % len(chains)]
                        nc.tensor.matmul(
                            ps[bb, m, n][:], zz[:],
                            wo[0][:, n * 512:(n + 1) * 512],
                            start=(pr == 0), stop=(pr == 0))
                    for ci, (bb, m, n) in enumerate(chains):
                        for ki in range(4):
                            k = 2 * ki + pr
                            cs = (bb * 4 + k // 2) * QS
                            nc.tensor.matmul(
                                ps[bb, m, n][:],
                                aop[k % 2][:, cs + m * P:cs + (m + 1) * P],
                                wo[k][:, n * 512:(n + 1) * 512],
                                start=(pr == 0 and ki == 0),
                                stop=(pr == 1 and ki == 3))
                for bb in range(B):
                    for m in range(QS // P):
                        ys = yst.tile([P, D], F32, tag="ys", name="ys")
                        for n in range(2):
                            nc.vector.tensor_tensor(
                                ys[:, n * 512:(n + 1) * 512], ps[bb, m, n][:],
                                bob[:, n * 512:(n + 1) * 512], ADD)
                            # half-width output DMAs flow as each bias lands
                            nc.sync.dma_start(
                                d_y[bb, m * P:(m + 1) * P, n * 512:(n + 1) * 512],
                                ys[:, n * 512:(n + 1) * 512])

    nc.compile()
    return nc


def get_nc():
    if "nc" not in _CACHE:
        _CACHE["nc"] = _build_nc()
    return _CACHE["nc"]


def make_in_maps(x, Wq, bq, Wk, bk, Wv, bv, Wo, bo):
    import ml_dtypes
    bf16 = ml_dtypes.bfloat16

    x = np.asarray(x, dtype=np.float32)
    Wq, Wk, Wv, Wo = (np.asarray(w, dtype=np.float32) for w in (Wq, Wk, Wv, Wo))
    bq, bk, bv, bo = (np.asarray(v, dtype=np.float32) for v in (bq, bk, bv, bo))
    scale = 1.0 / np.sqrt(np.float32(HD))
    woT = np.ascontiguousarray(Wo.T).astype(bf16)
    in_maps = []
    for c in range(CORES):
        b, g = c // 4, c % 4
        sl = slice(g * LD, (g + 1) * LD)
        xw = np.concatenate(
            [x[b].T, Wq[sl, :].T * scale, Wk[sl, :].T, Wv[sl, :].T],
            axis=1).astype(bf16)
        bqs = (bq[sl] * scale).reshape(2, P)
        bks = bk[sl].reshape(2, P)
        bqk = np.stack([bqs[0], bqs[1], bks[0], bks[1]], axis=1)
        bqk = np.ascontiguousarray(bqk, dtype=np.float32)
        vibo = np.empty((1, GH * VW + D), dtype=np.float32)
        for h in range(GH):
            vibo[0, h * VW: h * VW + HD] = bv[g * LD + h * HD:
                                              g * LD + (h + 1) * HD]
            vibo[0, h * VW + HD] = 1.0
        vibo[0, GH * VW:] = bo
        in_maps.append({
            "xw": xw, "bqk": bqk, "vibo": vibo, "woT": woT,
        })
    return in_maps


def assemble(results):
    out = np.empty((B, S, D), dtype=np.float32)
    for c in range(CORES):
        out[:, c * QS:(c + 1) * QS, :] = results[c]["y"]
    return out


def kernel(**inputs):
    from concourse.bass_utils import run_bass_kernel_spmd

    nc = get_nc()
    in_maps = make_in_maps(**inputs)
    res = run_bass_kernel_spmd(nc, in_maps, list(range(CORES)), trace=False)
    return assemble(res.results)


# revision 50
# speedup vs baseline: 1.0011x; 1.0011x over previous
"""MHA (B=2, S=2048, D=1024, H=16) on 8 Trainium2 NeuronCores.

Sharding: core c = (batch b = c//4, head-group g = c%4, 4 heads each).
Per core: Q/K/V projections for its 4 heads (tensor-parallel column split),
full attention for those heads in transposed (scoresT = [k, q]) layout so no
on-chip transposes are needed, then a per-head 8-way AllToAll redistributes
outputs so each core owns a 256-row sequence slice of BOTH batches with all
1024 concat-head dims, and runs the output projection for its slice. Host
only concatenates slices.

Schedule: one packed DRAM tensor [xT | wq*scale | wk | wv] streams in as 8
k-slabs; Q/K projections for heads 0/1 accumulate k-outer in PSUM while the
slabs arrive, so attention starts ~20us in. The V projection and the
heads-2/3 Q/K projections run as PE fillers inside the (ACT-engine-bound)
attention pipeline, spread so each phase stays balanced. Everything computes
in bf16 (PSUM accumulation fp32); softmax skips max-subtraction and folds
the row-sum into the attn@V matmul via an all-ones column on V. The AllToAll
payload is bf16; the out-projection pre-accumulates pair-0 k-tiles while the
last AllToAll is in flight.
"""

import numpy as np

B, S, D, H = 2, 2048, 1024, 16
HD = D // H          # 64
G = H // 4           # 4 head-groups
GH = 4               # heads per core
LD = GH * HD         # 256 local dims per core
CORES = 8
QS = S // CORES      # 256: per-core final sequence slice (per batch)
P = 128
KT = D // P          # 8 k-tiles of the model dim
VW = HD + 1          # 65: per-head V width incl. ones column
XW = S + 3 * LD      # 2816: slab row = xT row | wq | wk | wv
HQ = S // 2          # 1024: q-half processed per PSUM accumulator

_CACHE = {}


def _build_nc():
    import concourse.mybir as mybir
    import concourse.tile as tile
    from concourse import bacc

    F32 = mybir.dt.float32
    BF16 = mybir.dt.bfloat16
    EXP = mybir.ActivationFunctionType.Exp
    MUL = mybir.AluOpType.mult
    ADD = mybir.AluOpType.add

    nc = bacc.Bacc("TRN2", target_bir_lowering=False, debug=False,
                   num_devices=CORES)

    d_xw = nc.dram_tensor("xw", [D, XW], BF16, kind="ExternalInput")
    d_bqk = nc.dram_tensor("bqk", [P, 4], F32, kind="ExternalInput")
    d_vibo = nc.dram_tensor("vibo", [1, GH * VW + D], F32, kind="ExternalInput")
    d_wo = nc.dram_tensor("woT", [D, D], BF16, kind="ExternalInput")
    d_y = nc.dram_tensor("y", [B, QS, D], F32, kind="ExternalOutput")

    # slab column offsets
    OQ, OK, OV = S, S + LD, S + 2 * LD

    with tile.TileContext(nc) as tc:
        with (
            tc.tile_pool(name="statics", bufs=1) as st,
            tc.tile_pool(name="dram", bufs=1, space="DRAM") as dram,
        ):
            # zeros tile: lhsT of warm-up / warm-keeping no-op matmuls
            zz = st.tile([P, P], BF16, tag="zz", name="zz")
            nc.gpsimd.memset(zz[:], 0.0)
            # dummy activation so the ACT table load happens at t~1us, not
            # right before the first real exp
            dum = st.tile([1, 4], F32, tag="dum", name="dum")
            nc.scalar.activation(dum[:], zz[0:1, 0:4], EXP)

            xw = [st.tile([P, XW], BF16, tag=f"xw{k}", name=f"xw{k}")
                  for k in range(KT)]
            for k in range(KT):
                nc.sync.dma_start(xw[k][:], d_xw[k * P:(k + 1) * P, :])
            bqk = st.tile([P, 4], F32, tag="bqk", name="bqk")
            nc.sync.dma_start(bqk[:], d_bqk[:])

            vibo = st.tile([1, GH * VW + D], F32, tag="vibo", name="vibo")
            nc.sync.dma_start(vibo[:], d_vibo[:])
            wo = [st.tile([P, D], BF16, tag=f"wo{k}", name=f"wo{k}")
                  for k in range(KT)]
            for k in range(KT):
                nc.sync.dma_start(wo[k][:], d_wo[k * P:(k + 1) * P, :])

            vib = st.tile([P, GH * VW], F32, tag="vib", name="vib")
            nc.gpsimd.partition_broadcast(vib[:], vibo[:, 0:GH * VW])
            bob = st.tile([P, D], F32, tag="bob", name="bob")

            # persistent activations (m index: 0 = heads 0/1, 1 = heads 2/3)
            qm = [st.tile([P, S], BF16, tag=f"qm{t}", name=f"qm{t}") for t in range(2)]
            km = [st.tile([P, S], BF16, tag=f"km{t}", name=f"km{t}") for t in range(2)]
            vaug = [st.tile([P, GH * VW], BF16, tag=f"va{s}", name=f"va{s}")
                    for s in range(S // P)]
            outT = [st.tile([P, S], BF16, tag=f"oT{t}", name=f"oT{t}") for t in range(2)]
            aop = [st.tile([P, CORES * QS], BF16, tag=f"aop{p}", name=f"aop{p}")
                   for p in range(2)]

            a_ins = [dram.tile([CORES * HD, QS], BF16, name=f"a_in{h}")
                     for h in range(GH)]
            a_outs = [dram.tile([CORES * HD, QS], BF16, name=f"a_out{h}")
                      for h in range(GH)]

            # ---- Q/K m0 (heads 0/1) + V s0 projections, k-outer over the
            # streaming slabs (8 PSUM banks: 4 q + 3 k + 1 v; K m0 n3 runs
            # later as a filler, it isn't consumed until head-0 step 12) ----
            with tc.tile_pool(name="pm0", bufs=1, space="PSUM") as pm0:
                qn = [pm0.tile([P, 512], F32, tag=f"qn{n}", name=f"qn{n}")
                      for n in range(4)]
                kn = [pm0.tile([P, 512], F32, tag=f"kn{n}", name=f"kn{n}")
                      for n in range(3)]
                v0 = pm0.tile([P, LD], F32, tag="v0", name="v0")

                def warm(count):
                    # zero-matmuls ramp the PE pstate / keep the busy streak
                    # alive between slab arrivals (results never read)
                    for _ in range(count):
                        nc.tensor.matmul(qn[0][:, 0:P], zz[:], zz[:],
                                         start=True, stop=True)

                warm(60)
                for k in range(KT):
                    for n in range(4):
                        nc.tensor.matmul(
                            qn[n][:], xw[k][:, OQ:OQ + P],
                            xw[k][:, n * 512:(n + 1) * 512],
                            start=(k == 0), stop=(k == KT - 1))
                        if n < 3:
                            nc.tensor.matmul(
                                kn[n][:], xw[k][:, OK:OK + P],
                                xw[k][:, n * 512:(n + 1) * 512],
                                start=(k == 0), stop=(k == KT - 1))
                    nc.tensor.matmul(
                        v0[:], xw[k][:, 0:P], xw[k][:, OV:OV + LD],
                        start=(k == 0), stop=(k == KT - 1))
                # evac on 3 parallel lanes (ACT is idle until attention):
                # DVE k1,q0,q1 + v0; ACT k0,q2,q3 (Identity+bias); Pool k2
                IDENT = mybir.ActivationFunctionType.Identity
                nc.vector.tensor_tensor(
                    km[0][:, 512:1024], kn[1][:],
                    bqk[:, 2:3].to_broadcast((P, 512)), ADD)
                for n in range(2):
                    nc.vector.tensor_tensor(
                        qm[0][:, n * 512:(n + 1) * 512], qn[n][:],
                        bqk[:, 0:1].to_broadcast((P, 512)), ADD)
                nc.scalar.activation(km[0][:, 0:512], kn[0][:], IDENT,
                                     bias=bqk[:, 2:3])
                nc.scalar.activation(km[0][:, 1024:1536], kn[2][:], IDENT,
                                     bias=bqk[:, 2:3])
                for n in (2, 3):
                    nc.scalar.activation(
                        qm[0][:, n * 512:(n + 1) * 512], qn[n][:], IDENT,
                        bias=bqk[:, 0:1])
                va = vaug[0].rearrange("p (h w) -> p h w", w=VW)
                vb = vib.rearrange("p (h w) -> p h w", w=VW)
                nc.vector.tensor_tensor(
                    va[:, :, 0:HD], v0.rearrange("p (h w) -> p h w", w=HD),
                    vb[:, :, 0:HD], ADD)
                nc.vector.tensor_copy(va[:, :, HD:VW], vb[:, :, HD:VW])

            # ---- attention (ACT-bound), with V and Q/K m1 as PE fillers ----
            with (
                tc.tile_pool(name="psc", bufs=2, space="PSUM") as psc,
                tc.tile_pool(name="pav", bufs=1, space="PSUM") as pav,
                tc.tile_pool(name="pj", bufs=1, space="PSUM") as pj,
                tc.tile_pool(name="exp", bufs=4) as xp,
                tc.tile_pool(name="nrm", bufs=2) as nr,
            ):
                def v_chain(s):
                    vt = pv.tile([P, LD], F32, tag="vt", name="vt")
                    for k in range(KT):
                        nc.tensor.matmul(
                            vt[:], xw[k][:, s * P:(s + 1) * P],
                            xw[k][:, OV:OV + LD],
                            start=(k == 0), stop=(k == KT - 1))
                    va = vaug[s].rearrange("p (h w) -> p h w", w=VW)
                    vb = vib.rearrange("p (h w) -> p h w", w=VW)
                    nc.vector.tensor_tensor(
                        va[:, :, 0:HD], vt.rearrange("p (h w) -> p h w", w=HD),
                        vb[:, :, 0:HD], ADD)
                    nc.vector.tensor_copy(va[:, :, HD:VW], vb[:, :, HD:VW])

                # q/k projection chains, emitted as two 4-matmul chunks
                # sharing one PSUM accumulation group
                _jt = {}

                def qk_chunk(which, m, n, c):
                    dst, boff, off = ((qm, m, OQ) if which == 0 else (km, 2 + m, OK))
                    if c == 0:
                        _jt[which, m, n] = pj.tile([P, 512], F32, tag="jt", name="jt")
                    jt = _jt[which, m, n]
                    for k in range(c * 4, c * 4 + 4):
                        nc.tensor.matmul(
                            jt[:], xw[k][:, off + m * P:off + (m + 1) * P],
                            xw[k][:, n * 512:(n + 1) * 512],
                            start=(k == 0), stop=(k == KT - 1))
                    if c == 1:
                        nc.vector.tensor_tensor(
                            dst[m][:, n * 512:(n + 1) * 512], jt[:],
                            bqk[:, boff:boff + 1].to_broadcast((P, 512)), ADD)

                # filler schedule: (h, half, kb) -> list of emit callables.
                # V chains pace head-0 half-0 (vaug[s] ready 2 steps early);
                # k-m0-n3 (consumed from head-0 step 12) lands on steps 0/1;
                # q1n0/q1n1/k1n0 must precede head 2; the rest slots into
                # head 2's ACT-bound slack just before first use.
                sched = {}
                for s in range(2, S // P):
                    sched.setdefault((0, 0, s - 2), []).append(lambda s=s: v_chain(s))
                sched.setdefault((0, 0, 0), []).append(lambda: v_chain(1))
                for c in range(2):
                    sched.setdefault((0, 0, 4 + c), []).append(
                        lambda c=c: qk_chunk(1, 0, 3, c))
                # q1n0/q1n1/k1n0 precede head 2; head 1 is ACT-bound with
                # ~240ns/step of PE slack, so spread the chunks there (the
                # LAG-deep pipeline absorbs them without starving the ACT)
                pre2 = [(0, n, c) for n in range(2) for c in range(2)] + \
                       [(1, 0, 0), (1, 0, 1)]
                for i, (w, n, c) in enumerate(pre2):
                    sched.setdefault((1, 0, 2 * i), []).append(
                        lambda w=w, n=n, c=c: qk_chunk(w, 1, n, c))
                in2 = [(1, 1, 1, 1, 1, 0), (1, 1, 3, 1, 1, 1),
                       (2, 0, 4, 1, 2, 0), (2, 0, 5, 1, 2, 1),
                       (2, 0, 8, 1, 3, 0), (2, 0, 9, 1, 3, 1),
                       (2, 0, 10, 0, 2, 0), (2, 0, 11, 0, 2, 1),
                       (2, 0, 13, 0, 3, 0), (2, 0, 14, 0, 3, 1)]
                for (h, half, kb, w, n, c) in in2:
                    sched.setdefault((h, half, kb), []).append(
                        lambda w=w, n=n, c=c: qk_chunk(w, 1, n, c))

                NKB = S // P
                av_t = {}

                def finish_half(h, half):
                    # normalize: outT_h = av[0:64] / av[64], then stage + A2A
                    ht, hp = h // 2, (h % 2) * HD
                    q0 = half * HQ
                    av = av_t[h, half]
                    last = (h == GH - 1 and half == 1)
                    if not last:
                        # quick evac frees the accumulator banks fast
                        avs = nr.tile([VW, HQ], F32, tag="avs", name="avs")
                        nc.vector.tensor_copy(avs[:], av[:])
                        rr = nr.tile([1, HQ], F32, tag="rr", name="rr")
                        nc.vector.reciprocal(rr[:], avs[HD:VW, :])
                        rrb = nr.tile([HD, HQ], F32, tag="rrb", name="rrb")
                        nc.gpsimd.partition_broadcast(rrb[:], rr[:])
                        nc.vector.tensor_tensor(
                            outT[ht][hp:hp + HD, q0:q0 + HQ],
                            avs[0:HD, :], rrb[:], MUL)
                        # stage this half for the A2A: q columns [q0, q0+HQ)
                        # are the slices of cores 4*half..3+4*half = a_in rows
                        nc.sync.dma_start(
                            a_ins[h][half * 4 * HD:(half + 1) * 4 * HD, :]
                            .rearrange("(j p) c -> p j c", p=HD),
                            outT[ht][hp:hp + HD, q0:q0 + HQ]
                            .rearrange("p (j c) -> p j c", c=QS))
                    else:
                        # latency-critical path into the final A2A: both
                        # recips back-to-back on DVE, broadcasts pipeline on
                        # Pool, then muls + per-chunk stage DMAs
                        rrs, rcs = [], []
                        for cc in range(2):
                            rr = nr.tile([1, 512], F32, tag="rr", name="rr")
                            nc.vector.reciprocal(
                                rr[:], av[HD:VW, cc * 512:(cc + 1) * 512])
                            rrs.append(rr)
                        for cc in range(2):
                            rc = nr.tile([HD, 512], F32, tag="rc", name="rc")
                            nc.gpsimd.partition_broadcast(rc[:], rrs[cc][:])
                            rcs.append(rc)
                        for cc in range(2):
                            c0 = q0 + cc * 512
                            nc.vector.tensor_tensor(
                                outT[ht][hp:hp + HD, c0:c0 + 512],
                                av[0:HD, cc * 512:(cc + 1) * 512],
                                rcs[cc][:], MUL)
                            # j-blocks 2 per 512-col chunk
                            j0 = half * 4 + cc * 2
                            nc.sync.dma_start(
                                a_ins[h][j0 * HD:(j0 + 2) * HD, :]
                                .rearrange("(j p) c -> p j c", p=HD),
                                outT[ht][hp:hp + HD, c0:c0 + 512]
                                .rearrange("p (j c) -> p j c", c=QS))
                    if half == 1:
                        # per-head 8-way AllToAll (overlaps later heads)
                        nc.gpsimd.collective_compute(
                            "AllToAll",
                            mybir.AluOpType.bypass,
                            replica_groups=[list(range(CORES))],
                            ins=[a_ins[h][:]],
                            outs=[a_outs[h][:]],
                        )
                        if h == 1 or h == 3:
                            p = h // 2
                            for hh in range(2):
                                nc.sync.dma_start(
                                    aop[p][hh * HD:(hh + 1) * HD, :].rearrange(
                                        "p (j c) -> p j c", c=QS),
                                    a_outs[2 * p + hh].rearrange(
                                        "(j p) c -> p j c", p=HD))

                def av_mm(h, half, kb, ex):
                    if kb == 0:
                        av_t[h, half] = pav.tile([VW, HQ], F32, tag="av", name="av")
                    av = av_t[h, half]
                    for nn in range(2):
                        nc.tensor.matmul(
                            av[:, nn * 512:(nn + 1) * 512],
                            vaug[kb][:, h * VW:(h + 1) * VW],
                            ex[:, nn * 512:(nn + 1) * 512],
                            start=(kb == 0), stop=(kb == NKB - 1))
                    if kb == NKB - 1:
                        finish_half(h, half)

                # one flat software pipeline across every (head, half, kb):
                # sc/exp run LAG steps ahead of attn@V, so the ACT engine
                # never stalls, including across half/head boundaries where
                # the normalization chain overlaps the next half's scores.
                LAG = 1
                with tc.tile_pool(name="pv", bufs=1, space="PSUM") as pv:
                    exq = []
                    for h in range(GH):
                        ht, hp = h // 2, (h % 2) * HD
                        kTh = km[ht][hp:hp + HD, :]
                        qTh = qm[ht][hp:hp + HD, :]
                        for half in range(2):
                            q0 = half * HQ
                            for kb in range(NKB):
                                sc = psc.tile([P, HQ], F32, tag="sc", name="sc")
                                for nn in range(2):
                                    nc.tensor.matmul(
                                        sc[:, nn * 512:(nn + 1) * 512],
                                        kTh[:, kb * P:(kb + 1) * P],
                                        qTh[:, q0 + nn * 512:q0 + (nn + 1) * 512],
                                        start=True, stop=True)
                                for fn in sched.get((h, half, kb), ()):
                                    fn()
                                ex = xp.tile([P, HQ], BF16, tag="ex", name="ex")
                                nc.scalar.activation(ex[:], sc[:], EXP)
                                exq.append((h, half, kb, ex))
                                if len(exq) > LAG:
                                    av_mm(*exq.pop(0))
                    for entry in exq:
                        av_mm(*entry)

            # ---- out projection for my 256-row slice of each batch ----
            # k-tile k = dims [k*128,(k+1)*128) of batch bb = sender core
            # c' = bb*4 + k//2, its local head pair p = k%2; all 8 chains
            # pre-accumulate their pair-0 k-tiles while pair 1's A2A flies.
            with (
                tc.tile_pool(name="pop", bufs=1, space="PSUM") as pop,
                tc.tile_pool(name="yst", bufs=2) as yst,
            ):
                nc.gpsimd.partition_broadcast(bob[:], vibo[:, GH * VW:])
                chains = [(bb, m, n) for bb in range(B)
                          for m in range(QS // P) for n in range(2)]
                ps = {c: pop.tile([P, 512], F32, tag=f"ps{i}", name=f"ps{i}")
                      for i, c in enumerate(chains)}
                PAD = 70  # warm-keeping no-op matmuls while pair-1's A2A flies
                PRE = 0  # ramp the PE before the pair-0 accumulation
                for pr in range(2):
                    for i in range(PRE if pr == 0 else PAD):
                        bb, m, n = chains[i % len(chains)]
                        nc.tensor.matmul(
                            ps[bb, m, n][:], zz[:],
                            wo[0][:, n * 512:(n + 1) * 512],
                            start=(pr == 0), stop=(pr == 0))
                    for ci, (bb, m, n) in enumerate(chains):
                        for ki in range(4):
                            k = 2 * ki + pr
                            cs = (bb * 4 + k // 2) * QS
                            nc.tensor.matmul(
                                ps[bb, m, n][:],
                                aop[k % 2][:, cs + m * P:cs + (m + 1) * P],
                                wo[k][:, n * 512:(n + 1) * 512],
                                start=(pr == 0 and ki == 0),
                                stop=(pr == 1 and ki == 3))
                for bb in range(B):
                    for m in range(QS // P):
                        ys = yst.tile([P, D], F32, tag="ys", name="ys")
                        for n in range(2):
                            nc.vector.tensor_tensor(
                                ys[:, n * 512:(n + 1) * 512], ps[bb, m, n][:],
                                bob[:, n * 512:(n + 1) * 512], ADD)
                            # half-width output DMAs flow as each bias lands
                            nc.sync.dma_start(
                                d_y[bb, m * P:(m + 1) * P, n * 512:(n + 1) * 512],
                                ys[:, n * 512:(n + 1) * 512])

    nc.compile()
    return nc


def get_nc():
    if "nc" not in _CACHE:
        _CACHE["nc"] = _build_nc()
    return _CACHE["nc"]


def make_in_maps(x, Wq, bq, Wk, bk, Wv, bv, Wo, bo):
    import ml_dtypes
    bf16 = ml_dtypes.bfloat16

    x = np.asarray(x, dtype=np.float32)
    Wq, Wk, Wv, Wo = (np.asarray(w, dtype=np.float32) for w in (Wq, Wk, Wv, Wo))
    bq, bk, bv, bo = (np.asarray(v, dtype=np.float32) for v in (bq, bk, bv, bo))
    scale = 1.0 / np.sqrt(np.float32(HD))
    woT = np.ascontiguousarray(Wo.T).astype(bf16)
    in_maps = []
    for c in range(CORES):
        b, g = c // 4, c % 4
        sl = slice(g * LD, (g + 1) * LD)
        xw = np.concatenate(
            [x[b].T, Wq[sl, :].T * scale, Wk[sl, :].T, Wv[sl, :].T],
            axis=1).astype(bf16)
        bqs = (bq[sl] * scale).reshape(2, P)
        bks = bk[sl].reshape(2, P)
        bqk = np.stack([bqs[0], bqs[1], bks[0], bks[1]], axis=1)
        bqk = np.ascontiguousarray(bqk, dtype=np.float32)
        vibo = np.empty((1, GH * VW + D), dtype=np.float32)
        for h in range(GH):
            vibo[0, h * VW: h * VW + HD] = bv[g * LD + h * HD:
                                              g * LD + (h + 1) * HD]
            vibo[0, h * VW + HD] = 1.0
        vibo[0, GH * VW:] = bo
        in_maps.append({
            "xw": xw, "bqk": bqk, "vibo": vibo, "woT": woT,
        })
    return in_maps


def assemble(results):
    out = np.empty((B, S, D), dtype=np.float32)
    for c in range(CORES):
        out[:, c * QS:(c + 1) * QS, :] = results[c]["y"]
    return out


def kernel(**inputs):
    from concourse.bass_utils import run_bass_kernel_spmd

    nc = get_nc()
    in_maps = make_in_maps(**inputs)
    res = run_bass_kernel_spmd(nc, in_maps, list(range(CORES)), trace=False)
    return assemble(res.results)
